# revision 2
# baseline (speedup 1.0000x reference)
"""MixHop layer (hop0 + A@h1 + A^2@h2) on 8 trn2 NeuronCores — v4.

v2 base: 1D node partition across 8 cores (host-side edge-balancing
permutation), dense hop matmuls on TensorE, SpMM = SWDGE dma_gather of
neighbor features + one-hot scatter matmuls accumulating in PSUM,
halo via AllGather; h0 computed under AG1; interleaved-parity group
pipeline (AGs NOT overlapped with gather phases — both are HBM-bound
and a concurrent AG degrades ~3.6x, measured).

v4 changes vs v2:
 - Merged parity tables: hcat rows are [p0: h1|h2, p1: h1|h2] (1KB) and
   gathers use elem_step to pick the parity half — ONE AllGather per
   stage instead of two serialized ones (saves a floor + the gap).
 - Rank-1 bias folding: y1 = A@(xW1) + rowsum(A)*b1 (same for g/b2), so
   the dense phase is one fused 256-col matmul per window and the bias
   lands as a cheap rank-1 matmul in the DMA-bound SpMM phase.
 - single_packet=True gathers (~5% measured win).
 - Single DVE copy + single DMA per dense group (merged row layout).
"""
import heapq
import os
import sys

for p in ("/opt/trn_rl_repo", "/root/.axon_site/_ro/trn_rl_repo"):
    if os.path.isdir(p) and p not in sys.path:
        sys.path.append(p)

import numpy as np
import ml_dtypes

N = 50000
E = 600000
C = 128
CORES = 8
NW = 50                   # windows per core
RPC = NW * 128            # 6400 rows per core (padded)
NP = RPC * CORES          # 51200
SGS = int(os.environ.get("KM_SG", "2"))    # windows per supergroup
_SIZES = [1, 1] + [SGS] * NW
GROUPS = []
_w = 0
for _s in _SIZES:
    GROUPS.append((_w, min(NW, _w + _s)))
    _w += _s
    if _w >= NW:
        break
NQ = 4                    # SWDGE queues
LAG = int(os.environ.get("KM_LAG", "4"))   # groups of gather lookahead
HALVES = int(os.environ.get("KM_HALVES", "2"))  # calls per (group,parity)
SP = bool(int(os.environ.get("KM_SP", "1")))
PREP = int(os.environ.get("KM_PREP", "0"))  # prepare_only gather pipeline
NSEM = 32
DG = 5                    # dense-phase window batch

TRACE = False
_CACHE = {}


def _balance_perm(edge_row, edge_col):
    """relabel[old_row] = new_row (= core*RPC + window*128 + k), balancing
    per-(core,window) edge counts, then choosing each node's parity half
    (k<64 vs k>=64) to balance per-(core,window,parity) gather-chunk
    loads (the parity of a SOURCE node decides which gather bucket its
    out-edges land in; Bw takes a max over cores, so imbalance pads T)."""
    deg = np.bincount(edge_row, minlength=N).astype(np.int64)
    order = np.argsort(-deg, kind="stable")
    nslots = CORES * NW
    loads = [(0, s) for s in range(nslots)]
    heapq.heapify(loads)
    space = np.full(nslots, 128, np.int64)
    slot_of = np.empty(NP, np.int64)
    for r in order:
        while True:
            load, s = heapq.heappop(loads)
            if space[s] > 0:
                break
        space[s] -= 1
        slot_of[r] = s
        if space[s] > 0:
            heapq.heappush(loads, (load + deg[r], s))
    pad_nodes = np.arange(N, NP)
    pi = 0
    for s in range(nslots):
        for _ in range(space[s]):
            slot_of[pad_nodes[pi]] = s
            pi += 1

    # parity assignment: greedy over sources (high out-degree first),
    # pick the half that reduces the p0-p1 imbalance of the dest buckets
    # this node's out-edges hit.
    odeg = np.bincount(edge_col, minlength=N).astype(np.int64)
    eorder = np.argsort(edge_col, kind="stable")
    dst_slot = slot_of[edge_row[eorder]]
    starts = np.searchsorted(edge_col[eorder], np.arange(N + 1))
    imb = np.zeros(nslots, np.int64)
    cap = np.full((nslots, 2), 64, np.int64)
    par_of = np.zeros(NP, np.int64)
    for u in np.argsort(-odeg, kind="stable"):
        bs = dst_slot[starts[u]:starts[u + 1]]
        s = slot_of[u]
        sc = imb[bs].sum()
        if cap[s, 0] == 0:
            p = 1
        elif cap[s, 1] == 0:
            p = 0
        else:
            p = 1 if sc > 0 else 0
        par_of[u] = p
        if len(bs):
            np.add.at(imb, bs, 1 - 2 * p)
        cap[s, p] -= 1
    for u in pad_nodes:
        s = slot_of[u]
        p = 0 if cap[s, 0] > 0 else 1
        par_of[u] = p
        cap[s, p] -= 1

    # offsets within each (slot, parity) half
    new_of_old = np.empty(NP, np.int64)
    fill = np.zeros((nslots, 2), np.int64)
    for u in range(NP):
        s, p = slot_of[u], par_of[u]
        new_of_old[u] = s * 128 + p * 64 + fill[s, p]
        fill[s, p] += 1
    return new_of_old


def _build_plan(edge_row, edge_col, edge_val):
    relabel = _balance_perm(edge_row, edge_col)
    er = relabel[edge_row]
    ec = relabel[edge_col]

    core = er // RPC
    w = (er % RPC) // 128
    off = (er % 128).astype(np.int64)
    par = ((ec % 128) // 64).astype(np.int64)
    gidx = ((ec // 128) * 64 + (ec % 64)).astype(np.int16)

    gid = (core * NW + w) * 2 + par
    ngroups = CORES * NW * 2
    counts = np.bincount(gid, minlength=ngroups).reshape(CORES, NW, 2)
    Bw = np.maximum(1, ((counts.max(axis=0) + 127) // 128))  # [NW, 2]

    cstart = np.zeros((NW, 2), np.int64)
    calls = []
    cpos = 0
    for (w0, w1) in GROUPS:
        for p in (0, 1):
            ws = list(range(w0, w1))
            nch = int(Bw[w0:w1, p].sum())
            for wi in ws:
                cstart[wi, p] = cpos
                cpos += int(Bw[wi, p])
            calls.append(dict(par=p, ws=ws, cstart=cpos - nch, nch=nch))
    T = cpos

    order = np.argsort(gid, kind="stable")
    gs = np.zeros(ngroups + 1, np.int64)
    np.cumsum(counts.reshape(-1), out=gs[1:])
    rank = np.arange(E, dtype=np.int64) - gs[gid[order]]
    pos = cstart[w[order], par[order]] * 128 + rank
    flat = core[order] * (T * 128) + pos

    idx_p = np.zeros(CORES * T * 128, np.int16)
    idx_p[flat] = gidx[order]
    idx_p = idx_p.reshape(CORES, T, 128)

    pt = np.zeros((CORES * T * 128, 128), ml_dtypes.bfloat16)
    pt[flat, off[order]] = edge_val[order].astype(ml_dtypes.bfloat16)
    pt = pt.reshape(CORES, T, 128, 128).transpose(0, 2, 1, 3)
    pt = np.ascontiguousarray(pt.reshape(CORES, 128, T * 128))

    seg = idx_p.reshape(CORES, T * 128 // 16, 16)
    wrapped16 = seg.transpose(0, 2, 1)
    gidx_w = np.ascontiguousarray(np.tile(wrapped16, (1, 8, 1)))

    rs = np.zeros(NP, np.float32)
    np.add.at(rs, er, edge_val)
    rs_tab = rs.reshape(CORES, NW * 128)

    return dict(Bw=Bw, cstart=cstart, calls=calls, T=T,
                pt=pt, gidx_w=gidx_w, relabel=relabel, rs_tab=rs_tab)


def _build_program(plan):
    import concourse.bass as bass
    import concourse.bacc as bacc
    import concourse.mybir as mybir
    import concourse.tile as tile

    dt = mybir.dt
    Bw, cstart, calls, T = plan["Bw"], plan["cstart"], plan["calls"], plan["T"]

    nc = bacc.Bacc("TRN2", target_bir_lowering=False, debug=False,
                   num_devices=CORES, num_swdge_queues=NQ)

    xT_d = nc.dram_tensor("xT", [128, RPC], dt.bfloat16, kind="ExternalInput")
    wb_d = nc.dram_tensor("wb", [128, 768], dt.bfloat16, kind="ExternalInput")
    rs_d = nc.dram_tensor("rs", [1, NW * 128], dt.bfloat16,
                          kind="ExternalInput")
    pt_d = nc.dram_tensor("ptt", [128, T * 128], dt.bfloat16,
                          kind="ExternalInput")
    gix_d = nc.dram_tensor("gixt", [128, T * 8], dt.int16,
                           kind="ExternalInput")
    out0_d = nc.dram_tensor("out0", [128, NW, 128], dt.bfloat16,
                            kind="ExternalOutput")
    out1_d = nc.dram_tensor("out1", [128, NW, 128], dt.bfloat16,
                            kind="ExternalOutput")
    out2_d = nc.dram_tensor("out2", [128, NW, 128], dt.bfloat16,
                            kind="ExternalOutput")

    qn = [0]
    dma_sems = None
    sem_uses = [0] * NSEM
    q_dirty = [[] for _ in range(NQ)]
    if PREP:
        dma_sems = [nc.alloc_semaphore(f"gdma{s}") for s in range(NSEM)]

    with tile.TileContext(nc) as tc:
        with (
            tc.tile_pool(name="const", bufs=1) as constp,
            tc.tile_pool(name="gath", bufs=8) as gathp,
            tc.tile_pool(name="pt", bufs=2) as ptp,
            tc.tile_pool(name="ev", bufs=4) as evp,
            tc.tile_pool(name="psum", bufs=6, space="PSUM") as psp,
            tc.tile_pool(name="psd", bufs=2, space="PSUM") as psdp,
            tc.tile_pool(name="dram", bufs=1, space="DRAM") as dramp,
        ):
            xT = constp.tile([128, RPC], dt.bfloat16)
            nc.sync.dma_start(xT[:], xT_d[:])
            wb = constp.tile([128, 768], dt.bfloat16)
            nc.sync.dma_start(wb[:], wb_d[:])
            rs = constp.tile([1, NW * 128], dt.bfloat16)
            nc.sync.dma_start(rs[:], rs_d[:])
            gixt = constp.tile([128, T * 8], dt.int16)
            nc.sync.dma_start(gixt[:], gix_d[:])
            ones = constp.tile([1, 128], dt.bfloat16)
            nc.vector.memset(ones[:], 1.0)
            if PREP:
                for s in range(NSEM):
                    nc.gpsimd.sem_clear(dma_sems[s])

            # merged-parity halo tables: row j of window w holds
            # [p0(off j%64): h1|h2 | p1(off 64+j%64): h1|h2]  (1 KiB rows)
            hcat_sh = dramp.tile([RPC // 2, 512], dt.bfloat16, name="hsh")
            hcat_fl = dramp.tile([NP // 2, 512], dt.bfloat16,
                                 addr_space="Shared", name="hfl")
            g_sh = dramp.tile([RPC // 2, 256], dt.bfloat16, name="gsh")
            g_fl = dramp.tile([NP // 2, 256], dt.bfloat16,
                              addr_space="Shared", name="gfl")

            # ---- dense h1|h2 phase (one fused matmul per window) ----
            for w0 in range(0, NW, DG):
                nwg = min(DG, NW - w0)
                hb = evp.tile([128, nwg, 256], dt.bfloat16, tag="hb")
                for wl in range(nwg):
                    w = w0 + wl
                    ph = psp.tile([128, 256], dt.float32, tag="ps")
                    nc.tensor.matmul(ph[:], xT[:, w * 128:(w + 1) * 128],
                                     wb[:, 128:384], start=True, stop=True)
                    nc.vector.tensor_copy(hb[:, wl, :], ph[:])
                # partition (p*64+a) -> row (w0+g)*64+a, cols p*256
                for par in (0, 1):
                    hv = hcat_sh[w0 * 64:(w0 + nwg) * 64,
                                 par * 256:(par + 1) * 256].rearrange(
                        "(g a) c -> a g c", a=64)
                    nc.sync.dma_start(hv[:],
                                      hb[par * 64:(par + 1) * 64, :, :])

            # ---- AG1 (single merged collective) ----
            ov1 = hcat_fl[:].rearrange("(c r) f -> c r f", c=CORES)
            nc.gpsimd.collective_compute(
                "AllGather", mybir.AluOpType.bypass,
                replica_groups=[list(range(CORES))],
                ins=[hcat_sh[:].opt()],
                outs=[ov1.opt()])

            # ---- h0 phase (overlaps AG1) ----
            for w0 in range(0, NW, DG):
                nwg = min(DG, NW - w0)
                h0b = evp.tile([128, nwg, 128], dt.bfloat16, tag="h0")
                for wl in range(nwg):
                    w = w0 + wl
                    ph0 = psdp.tile([128, 128], dt.float32, tag="ph0",
                                    bufs=2)
                    nc.tensor.matmul(ph0[:], ones[:], wb[0:1, 384:512],
                                     start=True, stop=False)
                    nc.tensor.matmul(ph0[:], xT[:, w * 128:(w + 1) * 128],
                                     wb[:, 0:128], start=False, stop=True)
                    nc.vector.tensor_copy(h0b[:, wl, :], ph0[:])
                nc.sync.dma_start(out0_d[:, w0:w0 + nwg, :], h0b[:])

            def spmm_pass(src_fl, elem, out_cols, oud, evict_g,
                          pass_no, ag_after=None):
                pend = {}

                def issue_gather(gi, p):
                    call = calls[gi * 2 + p]
                    nch = call["nch"]
                    cs = call["cstart"]
                    gt = gathp.tile([128, nch, elem], dt.bfloat16,
                                    tag=f"g{p}", name=f"gt{p}",
                                    bufs=LAG + 3 if p == 0 else LAG + 2)
                    bounds = [round(i * nch / HALVES)
                              for i in range(HALVES + 1)]
                    for (a, b) in zip(bounds, bounds[1:]):
                        if b <= a:
                            continue
                        nc.gpsimd.dma_gather(
                            gt[:, a:b, :],
                            src_fl[:, p * elem:(p + 1) * elem],
                            gixt[:, (cs + a) * 8:(cs + b) * 8],
                            num_idxs=(b - a) * 128,
                            num_idxs_reg=(b - a) * 128,
                            elem_size=elem, elem_step=2 * elem,
                            single_packet=SP, queue_num=qn[0] % NQ)
                        qn[0] += 1
                    return (gt, cs)

                def issue_ptt(gi):
                    c0 = calls[gi * 2]["cstart"]
                    c1 = calls[gi * 2 + 1]["cstart"] + calls[gi * 2 + 1]["nch"]
                    ptt = ptp.tile([128, (c1 - c0) * 128], dt.bfloat16,
                                   tag="ptt", name="ptt", bufs=3)
                    nc.scalar.dma_start(ptt[:], pt_d[:, c0 * 128:c1 * 128])
                    pend.setdefault(gi, {})["ptt"] = ptt

                def _process_group(gi):
                    w0, w1 = GROUPS[gi]
                    nwg = w1 - w0
                    c0 = calls[gi * 2]["cstart"]
                    gts = pend.pop(gi)
                    ptt = gts.pop("ptt")
                    if PREP:
                        for p in (0, 1):
                            for (si, rep) in gts[p][2]:
                                nc.tensor.wait_ge(dma_sems[si], 16 * rep)
                    ycb = evp.tile([128, nwg, 128], dt.bfloat16, tag="yc",
                                   name="ycb")
                    gcb = None
                    if evict_g:
                        gcb = evp.tile([128, nwg, 128], dt.bfloat16, tag="gc",
                                       name="gcb")
                    for w in range(w0, w1):
                        nchw = int(Bw[w, 0] + Bw[w, 1])
                        ps = psp.tile([128, out_cols], dt.float32, tag="ps")
                        k = 0
                        if pass_no == 1:
                            nc.tensor.matmul(ps[:],
                                             rs[:, w * 128:(w + 1) * 128],
                                             wb[0:1, 512:768],
                                             start=True, stop=False)
                            k = 1
                        for p in (0, 1):
                            gt, cs = gts[p][0], gts[p][1]
                            for bch in range(int(Bw[w, p])):
                                cg = int(cstart[w, p]) + bch
                                lp = cg - cs
                                nc.tensor.matmul(
                                    ps[:],
                                    ptt[:, (cg - c0) * 128:(cg - c0 + 1) * 128],
                                    gt[:, lp, :],
                                    start=(k == 0),
                                    stop=(p == 1 and bch == int(Bw[w, 1]) - 1))
                                k += 1
                        nc.vector.tensor_copy(ycb[:, w - w0, :], ps[:, 0:128])
                        if evict_g:
                            nc.vector.tensor_copy(gcb[:, w - w0, :],
                                                  ps[:, 128:256])
                    nc.sync.dma_start(oud[:, w0:w1, :], ycb[:])
                    if evict_g:
                        for par in (0, 1):
                            gv = g_sh[w0 * 64:w1 * 64,
                                      par * 128:(par + 1) * 128].rearrange(
                                "(g a) c -> a g c", a=64)
                            nc.scalar.dma_start(
                                gv[:], gcb[par * 64:(par + 1) * 64, :, :])

                def prep_call(gi, p):
                    # split like issue_gather: single_packet calls above
                    # ~1.5k idxs crash the device, and halves keep all 4
                    # queues fed
                    call = calls[gi * 2 + p]
                    nch = call["nch"]
                    cs = call["cstart"]
                    gt = gathp.tile([128, nch, elem], dt.bfloat16,
                                    tag=f"g{p}", name=f"gt{p}",
                                    bufs=LAG + 3 if p == 0 else LAG + 2)
                    sis = []
                    bounds = [round(i * nch / HALVES)
                              for i in range(HALVES + 1)]
                    for (a, b) in zip(bounds, bounds[1:]):
                        if b <= a:
                            continue
                        q = qn[0] % NQ
                        si = qn[0] % NSEM
                        qn[0] += 1
                        sem_uses[si] += 1
                        nc.gpsimd.dma_gather(
                            gt[:, a:b, :],
                            src_fl[:, p * elem:(p + 1) * elem],
                            gixt[:, (cs + a) * 8:(cs + b) * 8],
                            num_idxs=(b - a) * 128,
                            num_idxs_reg=(b - a) * 128,
                            elem_size=elem, elem_step=2 * elem,
                            single_packet=SP, queue_num=q,
                            prepare_only=True, sem=dma_sems[si])
                        q_dirty[q].append(gt)
                        sis.append((si, sem_uses[si]))
                    pend.setdefault(gi, {})[p] = (gt, cs, sis)

                def trigger_dirty(queues=None):
                    for q in (range(NQ) if queues is None else queues):
                        if q_dirty[q]:
                            nc.gpsimd.trigger_dma(count=None, queue_num=q)
                            q_dirty[q] = []

                nG = len(GROUPS)
                if PREP:
                    PD = LAG
                    for gj in range(min(PD, nG)):
                        prep_call(gj, 0)
                        prep_call(gj, 1)
                    for gj in range(min(3, nG)):
                        issue_ptt(gj)
                    for gi in range(nG):
                        # Pool is in-order: every issued prep has retired
                        # before a later trigger executes, so firing all
                        # dirty queues is safe.
                        trigger_dirty()
                        if gi + PD < nG:
                            prep_call(gi + PD, 0)
                            prep_call(gi + PD, 1)
                        _process_group(gi)
                        if ag_after and gi in ag_after:
                            ag_after[gi]()
                        if gi + 3 < nG:
                            issue_ptt(gi + 3)
                    return
                for gi in range(nG + LAG):
                    if gi < nG:
                        pend[gi] = {0: issue_gather(gi, 0)}
                    ok = gi - (LAG - 2)
                    if 0 <= ok < nG:
                        pend[ok] = pend.get(ok, {})
                        pend[ok][1] = issue_gather(ok, 1)
                        issue_ptt(ok)
                    # odd-stride bump so the parity->queue mapping rotates
                    # (4 calls/iter on 4 queues otherwise pins p0 to q0/q1)
                    qn[0] += 1
                    pk = gi - LAG
                    if 0 <= pk < nG:
                        _process_group(pk)
                        if ag_after and pk in ag_after:
                            ag_after[pk]()

            # AG2 fired right after the last group's g_sh write
            def ag2():
                ov2 = g_fl[:].rearrange("(c r) f -> c r f", c=CORES)
                nc.gpsimd.collective_compute(
                    "AllGather", mybir.AluOpType.bypass,
                    replica_groups=[list(range(CORES))],
                    ins=[g_sh[:].opt()],
                    outs=[ov2.opt()])

            spmm_pass(hcat_fl, 256, 256, out1_d, True, 1,
                      ag_after={len(GROUPS) - 1: ag2})
            spmm_pass(g_fl, 128, 128, out2_d, False, 2)

    nc.compile()
    return nc


def _prepare_inputs(x, W, b, plan):
    relabel = plan["relabel"]
    xpad = np.zeros((NP, C), np.float32)
    xpad[relabel[:N]] = x
    xT = xpad.T
    Wp = np.concatenate([W[0], W[1], W[2]], axis=1)
    biasrow = np.zeros((128, 384), np.float32)
    biasrow[0] = np.concatenate([b[0], b[1], b[2]])
    wb = np.concatenate([Wp, biasrow], axis=1)

    in_maps = []
    for c in range(CORES):
        in_maps.append({
            "xT": np.ascontiguousarray(
                xT[:, c * RPC:(c + 1) * RPC]).astype(ml_dtypes.bfloat16),
            "wb": wb.astype(ml_dtypes.bfloat16),
            "rs": plan["rs_tab"][c][None, :].astype(ml_dtypes.bfloat16),
            "ptt": plan["pt"][c],
            "gixt": plan["gidx_w"][c],
        })
    return in_maps


def kernel(x, W, b, edge_val, edge_row, edge_col):
    x = np.asarray(x, np.float32)
    W = np.asarray(W, np.float32)
    b = np.asarray(b, np.float32)
    edge_val = np.asarray(edge_val, np.float32)
    edge_row = np.asarray(edge_row, np.int32)
    edge_col = np.asarray(edge_col, np.int32)

    from concourse.bass_utils import run_bass_kernel_spmd

    key = hash((edge_row.tobytes(), edge_col.tobytes(), edge_val.tobytes()))
    if key not in _CACHE:
        plan = _build_plan(edge_row, edge_col, edge_val)
        nc = _build_program(plan)
        _CACHE[key] = (plan, nc)
    plan, nc = _CACHE[key]

    in_maps = _prepare_inputs(x, W, b, plan)
    res = run_bass_kernel_spmd(nc, in_maps, core_ids=list(range(CORES)),
                               trace=TRACE)
    kernel.last_results = res
    parts = []
    for c in range(CORES):
        r = res.results[c]
        blk = np.stack([np.asarray(r["out0"], np.float32),
                        np.asarray(r["out1"], np.float32),
                        np.asarray(r["out2"], np.float32)], axis=-2)
        parts.append(blk.transpose(1, 0, 2, 3).reshape(RPC, 384))
    full = np.concatenate(parts, axis=0)
    return np.ascontiguousarray(full[plan["relabel"][:N]])


kernel.last_results = None

if __name__ == "__main__":
    rng = np.random.default_rng(0)
    x = rng.standard_normal((N, C), dtype=np.float32)
    W = rng.standard_normal((3, C, C), dtype=np.float32) / np.sqrt(C)
    b = rng.standard_normal((3, C), dtype=np.float32) * 0.01
    ev = rng.random(E, dtype=np.float32)
    er = rng.integers(0, N, E, dtype=np.int32)
    ec = rng.integers(0, N, E, dtype=np.int32)
    out = kernel(x=x, W=W, b=b, edge_val=ev, edge_row=er, edge_col=ec)
    print(out.shape, out.dtype)


# revision 3
# speedup vs baseline: 1.4783x; 1.4783x over previous
"""MixHop layer (hop0 + A@h1 + A^2@h2) on 8 trn2 NeuronCores — v4.

v2 base: 1D node partition across 8 cores (host-side edge-balancing
permutation), dense hop matmuls on TensorE, SpMM = SWDGE dma_gather of
neighbor features + one-hot scatter matmuls accumulating in PSUM,
halo via AllGather; h0 computed under AG1; interleaved-parity group
pipeline (AGs NOT overlapped with gather phases — both are HBM-bound
and a concurrent AG degrades ~3.6x, measured).

v4 changes vs v2:
 - Merged parity tables: hcat rows are [p0: h1|h2, p1: h1|h2] (1KB) and
   gathers use elem_step to pick the parity half — ONE AllGather per
   stage instead of two serialized ones (saves a floor + the gap).
 - Rank-1 bias folding: y1 = A@(xW1) + rowsum(A)*b1 (same for g/b2), so
   the dense phase is one fused 256-col matmul per window and the bias
   lands as a cheap rank-1 matmul in the DMA-bound SpMM phase.
 - single_packet=True gathers (~5% measured win).
 - Parity-balanced host plan (source-parity greedy assignment cuts
   chunk padding, T 668->600), bf16 outputs (host converts to f32),
   queue-rotation bump, 6-buf shared PSUM pool.
"""
import heapq
import os
import sys

for p in ("/opt/trn_rl_repo", "/root/.axon_site/_ro/trn_rl_repo"):
    if os.path.isdir(p) and p not in sys.path:
        sys.path.append(p)

import numpy as np
import ml_dtypes

N = 50000
E = 600000
C = 128
CORES = 8
NW = 50                   # windows per core
RPC = NW * 128            # 6400 rows per core (padded)
NP = RPC * CORES          # 51200
SGS = int(os.environ.get("KM_SG", "2"))    # windows per supergroup
_SIZES = [1, 1] + [SGS] * NW
GROUPS = []
_w = 0
for _s in _SIZES:
    GROUPS.append((_w, min(NW, _w + _s)))
    _w += _s
    if _w >= NW:
        break
NQ = 4                    # SWDGE queues
LAG = int(os.environ.get("KM_LAG", "4"))   # groups of gather lookahead
HALVES = int(os.environ.get("KM_HALVES", "2"))  # calls per (group,parity)
SP = bool(int(os.environ.get("KM_SP", "1")))
PREP = int(os.environ.get("KM_PREP", "0"))  # prepare_only gather pipeline
NSEM = 32
DG = 5                    # dense-phase window batch

TRACE = False
_CACHE = {}


def _balance_perm(edge_row, edge_col):
    """relabel[old_row] = new_row (= core*RPC + window*128 + k), balancing
    per-(core,window) edge counts, then choosing each node's parity half
    (k<64 vs k>=64) to balance per-(core,window,parity) gather-chunk
    loads (the parity of a SOURCE node decides which gather bucket its
    out-edges land in; Bw takes a max over cores, so imbalance pads T)."""
    deg = np.bincount(edge_row, minlength=N).astype(np.int64)
    order = np.argsort(-deg, kind="stable")
    nslots = CORES * NW
    loads = [(0, s) for s in range(nslots)]
    heapq.heapify(loads)
    space = np.full(nslots, 128, np.int64)
    slot_of = np.empty(NP, np.int64)
    for r in order:
        while True:
            load, s = heapq.heappop(loads)
            if space[s] > 0:
                break
        space[s] -= 1
        slot_of[r] = s
        if space[s] > 0:
            heapq.heappush(loads, (load + deg[r], s))
    pad_nodes = np.arange(N, NP)
    pi = 0
    for s in range(nslots):
        for _ in range(space[s]):
            slot_of[pad_nodes[pi]] = s
            pi += 1

    # parity assignment: greedy over sources (high out-degree first),
    # pick the half that reduces the p0-p1 imbalance of the dest buckets
    # this node's out-edges hit.
    odeg = np.bincount(edge_col, minlength=N).astype(np.int64)
    eorder = np.argsort(edge_col, kind="stable")
    dst_slot = slot_of[edge_row[eorder]]
    starts = np.searchsorted(edge_col[eorder], np.arange(N + 1))
    imb = np.zeros(nslots, np.int64)
    cap = np.full((nslots, 2), 64, np.int64)
    par_of = np.zeros(NP, np.int64)
    for u in np.argsort(-odeg, kind="stable"):
        bs = dst_slot[starts[u]:starts[u + 1]]
        s = slot_of[u]
        sc = imb[bs].sum()
        if cap[s, 0] == 0:
            p = 1
        elif cap[s, 1] == 0:
            p = 0
        else:
            p = 1 if sc > 0 else 0
        par_of[u] = p
        if len(bs):
            np.add.at(imb, bs, 1 - 2 * p)
        cap[s, p] -= 1
    for u in pad_nodes:
        s = slot_of[u]
        p = 0 if cap[s, 0] > 0 else 1
        par_of[u] = p
        cap[s, p] -= 1

    # offsets within each (slot, parity) half
    new_of_old = np.empty(NP, np.int64)
    fill = np.zeros((nslots, 2), np.int64)
    for u in range(NP):
        s, p = slot_of[u], par_of[u]
        new_of_old[u] = s * 128 + p * 64 + fill[s, p]
        fill[s, p] += 1
    return new_of_old


def _build_plan(edge_row, edge_col, edge_val):
    relabel = _balance_perm(edge_row, edge_col)
    er = relabel[edge_row]
    ec = relabel[edge_col]

    core = er // RPC
    w = (er % RPC) // 128
    off = (er % 128).astype(np.int64)
    par = ((ec % 128) // 64).astype(np.int64)
    gidx = ((ec // 128) * 64 + (ec % 64)).astype(np.int16)

    gid = (core * NW + w) * 2 + par
    ngroups = CORES * NW * 2
    counts = np.bincount(gid, minlength=ngroups).reshape(CORES, NW, 2)
    Bw = np.maximum(1, ((counts.max(axis=0) + 127) // 128))  # [NW, 2]

    cstart = np.zeros((NW, 2), np.int64)
    calls = []
    cpos = 0
    for (w0, w1) in GROUPS:
        for p in (0, 1):
            ws = list(range(w0, w1))
            nch = int(Bw[w0:w1, p].sum())
            for wi in ws:
                cstart[wi, p] = cpos
                cpos += int(Bw[wi, p])
            calls.append(dict(par=p, ws=ws, cstart=cpos - nch, nch=nch))
    T = cpos

    order = np.argsort(gid, kind="stable")
    gs = np.zeros(ngroups + 1, np.int64)
    np.cumsum(counts.reshape(-1), out=gs[1:])
    rank = np.arange(E, dtype=np.int64) - gs[gid[order]]
    pos = cstart[w[order], par[order]] * 128 + rank
    flat = core[order] * (T * 128) + pos

    idx_p = np.zeros(CORES * T * 128, np.int16)
    idx_p[flat] = gidx[order]
    idx_p = idx_p.reshape(CORES, T, 128)

    pt = np.zeros((CORES * T * 128, 128), ml_dtypes.bfloat16)
    pt[flat, off[order]] = edge_val[order].astype(ml_dtypes.bfloat16)
    pt = pt.reshape(CORES, T, 128, 128).transpose(0, 2, 1, 3)
    pt = np.ascontiguousarray(pt.reshape(CORES, 128, T * 128))

    seg = idx_p.reshape(CORES, T * 128 // 16, 16)
    wrapped16 = seg.transpose(0, 2, 1)
    gidx_w = np.ascontiguousarray(np.tile(wrapped16, (1, 8, 1)))

    rs = np.zeros(NP, np.float32)
    np.add.at(rs, er, edge_val)
    rs_tab = rs.reshape(CORES, NW * 128)

    return dict(Bw=Bw, cstart=cstart, calls=calls, T=T,
                pt=pt, gidx_w=gidx_w, relabel=relabel, rs_tab=rs_tab)


def _build_program(plan):
    import concourse.bass as bass
    import concourse.bacc as bacc
    import concourse.mybir as mybir
    import concourse.tile as tile

    dt = mybir.dt
    Bw, cstart, calls, T = plan["Bw"], plan["cstart"], plan["calls"], plan["T"]

    nc = bacc.Bacc("TRN2", target_bir_lowering=False, debug=False,
                   num_devices=CORES, num_swdge_queues=NQ)

    xT_d = nc.dram_tensor("xT", [128, RPC], dt.bfloat16, kind="ExternalInput")
    wb_d = nc.dram_tensor("wb", [128, 768], dt.bfloat16, kind="ExternalInput")
    rs_d = nc.dram_tensor("rs", [1, NW * 128], dt.bfloat16,
                          kind="ExternalInput")
    pt_d = nc.dram_tensor("ptt", [128, T * 128], dt.bfloat16,
                          kind="ExternalInput")
    gix_d = nc.dram_tensor("gixt", [128, T * 8], dt.int16,
                           kind="ExternalInput")
    out0_d = nc.dram_tensor("out0", [128, NW, 128], dt.bfloat16,
                            kind="ExternalOutput")
    out1_d = nc.dram_tensor("out1", [128, NW, 128], dt.bfloat16,
                            kind="ExternalOutput")
    out2_d = nc.dram_tensor("out2", [128, NW, 128], dt.bfloat16,
                            kind="ExternalOutput")

    qn = [0]
    dma_sems = None
    sem_uses = [0] * NSEM
    q_dirty = [[] for _ in range(NQ)]
    if PREP:
        dma_sems = [nc.alloc_semaphore(f"gdma{s}") for s in range(NSEM)]

    with tile.TileContext(nc) as tc:
        with (
            tc.tile_pool(name="const", bufs=1) as constp,
            tc.tile_pool(name="gath", bufs=8) as gathp,
            tc.tile_pool(name="pt", bufs=2) as ptp,
            tc.tile_pool(name="ev", bufs=4) as evp,
            tc.tile_pool(name="psum", bufs=6, space="PSUM") as psp,
            tc.tile_pool(name="psd", bufs=2, space="PSUM") as psdp,
            tc.tile_pool(name="dram", bufs=1, space="DRAM") as dramp,
        ):
            xT = constp.tile([128, RPC], dt.bfloat16)
            nc.sync.dma_start(xT[:], xT_d[:])
            wb = constp.tile([128, 768], dt.bfloat16)
            nc.sync.dma_start(wb[:], wb_d[:])
            rs = constp.tile([1, NW * 128], dt.bfloat16)
            nc.sync.dma_start(rs[:], rs_d[:])
            gixt = constp.tile([128, T * 8], dt.int16)
            nc.sync.dma_start(gixt[:], gix_d[:])
            ones = constp.tile([1, 128], dt.bfloat16)
            nc.vector.memset(ones[:], 1.0)
            if PREP:
                for s in range(NSEM):
                    nc.gpsimd.sem_clear(dma_sems[s])

            # merged-parity halo tables: row j of window w holds
            # [p0(off j%64): h1|h2 | p1(off 64+j%64): h1|h2]  (1 KiB rows)
            hcat_sh = dramp.tile([RPC // 2, 512], dt.bfloat16, name="hsh")
            hcat_fl = dramp.tile([NP // 2, 512], dt.bfloat16,
                                 addr_space="Shared", name="hfl")
            g_sh = dramp.tile([RPC // 2, 256], dt.bfloat16, name="gsh")
            g_fl = dramp.tile([NP // 2, 256], dt.bfloat16,
                              addr_space="Shared", name="gfl")

            # ---- dense h1|h2 phase (one fused matmul per window) ----
            for w0 in range(0, NW, DG):
                nwg = min(DG, NW - w0)
                hb = evp.tile([128, nwg, 256], dt.bfloat16, tag="hb")
                for wl in range(nwg):
                    w = w0 + wl
                    ph = psp.tile([128, 256], dt.float32, tag="ps")
                    nc.tensor.matmul(ph[:], xT[:, w * 128:(w + 1) * 128],
                                     wb[:, 128:384], start=True, stop=True)
                    nc.vector.tensor_copy(hb[:, wl, :], ph[:])
                # partition (p*64+a) -> row (w0+g)*64+a, cols p*256
                for par in (0, 1):
                    hv = hcat_sh[w0 * 64:(w0 + nwg) * 64,
                                 par * 256:(par + 1) * 256].rearrange(
                        "(g a) c -> a g c", a=64)
                    nc.sync.dma_start(hv[:],
                                      hb[par * 64:(par + 1) * 64, :, :])

            # ---- AG1 (single merged collective) ----
            ov1 = hcat_fl[:].rearrange("(c r) f -> c r f", c=CORES)
            nc.gpsimd.collective_compute(
                "AllGather", mybir.AluOpType.bypass,
                replica_groups=[list(range(CORES))],
                ins=[hcat_sh[:].opt()],
                outs=[ov1.opt()])

            # ---- h0 phase (overlaps AG1) ----
            for w0 in range(0, NW, DG):
                nwg = min(DG, NW - w0)
                h0b = evp.tile([128, nwg, 128], dt.bfloat16, tag="h0")
                for wl in range(nwg):
                    w = w0 + wl
                    ph0 = psdp.tile([128, 128], dt.float32, tag="ph0",
                                    bufs=2)
                    nc.tensor.matmul(ph0[:], ones[:], wb[0:1, 384:512],
                                     start=True, stop=False)
                    nc.tensor.matmul(ph0[:], xT[:, w * 128:(w + 1) * 128],
                                     wb[:, 0:128], start=False, stop=True)
                    nc.vector.tensor_copy(h0b[:, wl, :], ph0[:])
                nc.sync.dma_start(out0_d[:, w0:w0 + nwg, :], h0b[:])

            def spmm_pass(src_fl, elem, out_cols, oud, evict_g,
                          pass_no, ag_after=None):
                pend = {}

                def issue_gather(gi, p):
                    call = calls[gi * 2 + p]
                    nch = call["nch"]
                    cs = call["cstart"]
                    gt = gathp.tile([128, nch, elem], dt.bfloat16,
                                    tag=f"g{p}", name=f"gt{p}",
                                    bufs=LAG + 3 if p == 0 else LAG + 2)
                    bounds = [round(i * nch / HALVES)
                              for i in range(HALVES + 1)]
                    for (a, b) in zip(bounds, bounds[1:]):
                        if b <= a:
                            continue
                        nc.gpsimd.dma_gather(
                            gt[:, a:b, :],
                            src_fl[:, p * elem:(p + 1) * elem],
                            gixt[:, (cs + a) * 8:(cs + b) * 8],
                            num_idxs=(b - a) * 128,
                            num_idxs_reg=(b - a) * 128,
                            elem_size=elem, elem_step=2 * elem,
                            single_packet=SP, queue_num=qn[0] % NQ)
                        qn[0] += 1
                    return (gt, cs)

                def issue_ptt(gi):
                    c0 = calls[gi * 2]["cstart"]
                    c1 = calls[gi * 2 + 1]["cstart"] + calls[gi * 2 + 1]["nch"]
                    ptt = ptp.tile([128, (c1 - c0) * 128], dt.bfloat16,
                                   tag="ptt", name="ptt", bufs=3)
                    nc.scalar.dma_start(ptt[:], pt_d[:, c0 * 128:c1 * 128])
                    pend.setdefault(gi, {})["ptt"] = ptt

                def _process_group(gi):
                    w0, w1 = GROUPS[gi]
                    nwg = w1 - w0
                    c0 = calls[gi * 2]["cstart"]
                    gts = pend.pop(gi)
                    ptt = gts.pop("ptt")
                    if PREP:
                        for p in (0, 1):
                            for (si, rep) in gts[p][2]:
                                nc.tensor.wait_ge(dma_sems[si], 16 * rep)
                    ycb = evp.tile([128, nwg, 128], dt.bfloat16, tag="yc",
                                   name="ycb")
                    gcb = None
                    if evict_g:
                        gcb = evp.tile([128, nwg, 128], dt.bfloat16, tag="gc",
                                       name="gcb")
                    for w in range(w0, w1):
                        nchw = int(Bw[w, 0] + Bw[w, 1])
                        ps = psp.tile([128, out_cols], dt.float32, tag="ps")
                        k = 0
                        if pass_no == 1:
                            nc.tensor.matmul(ps[:],
                                             rs[:, w * 128:(w + 1) * 128],
                                             wb[0:1, 512:768],
                                             start=True, stop=False)
                            k = 1
                        for p in (0, 1):
                            gt, cs = gts[p][0], gts[p][1]
                            for bch in range(int(Bw[w, p])):
                                cg = int(cstart[w, p]) + bch
                                lp = cg - cs
                                nc.tensor.matmul(
                                    ps[:],
                                    ptt[:, (cg - c0) * 128:(cg - c0 + 1) * 128],
                                    gt[:, lp, :],
                                    start=(k == 0),
                                    stop=(p == 1 and bch == int(Bw[w, 1]) - 1))
                                k += 1
                        nc.vector.tensor_copy(ycb[:, w - w0, :], ps[:, 0:128])
                        if evict_g:
                            nc.vector.tensor_copy(gcb[:, w - w0, :],
                                                  ps[:, 128:256])
                    nc.sync.dma_start(oud[:, w0:w1, :], ycb[:])
                    if evict_g:
                        for par in (0, 1):
                            gv = g_sh[w0 * 64:w1 * 64,
                                      par * 128:(par + 1) * 128].rearrange(
                                "(g a) c -> a g c", a=64)
                            nc.scalar.dma_start(
                                gv[:], gcb[par * 64:(par + 1) * 64, :, :])

                def prep_call(gi, p):
                    # split like issue_gather: single_packet calls above
                    # ~1.5k idxs crash the device, and halves keep all 4
                    # queues fed
                    call = calls[gi * 2 + p]
                    nch = call["nch"]
                    cs = call["cstart"]
                    gt = gathp.tile([128, nch, elem], dt.bfloat16,
                                    tag=f"g{p}", name=f"gt{p}",
                                    bufs=LAG + 3 if p == 0 else LAG + 2)
                    sis = []
                    bounds = [round(i * nch / HALVES)
                              for i in range(HALVES + 1)]
                    for (a, b) in zip(bounds, bounds[1:]):
                        if b <= a:
                            continue
                        q = qn[0] % NQ
                        si = qn[0] % NSEM
                        qn[0] += 1
                        sem_uses[si] += 1
                        nc.gpsimd.dma_gather(
                            gt[:, a:b, :],
                            src_fl[:, p * elem:(p + 1) * elem],
                            gixt[:, (cs + a) * 8:(cs + b) * 8],
                            num_idxs=(b - a) * 128,
                            num_idxs_reg=(b - a) * 128,
                            elem_size=elem, elem_step=2 * elem,
                            single_packet=SP, queue_num=q,
                            prepare_only=True, sem=dma_sems[si])
                        q_dirty[q].append(gt)
                        sis.append((si, sem_uses[si]))
                    pend.setdefault(gi, {})[p] = (gt, cs, sis)

                def trigger_dirty(queues=None):
                    for q in (range(NQ) if queues is None else queues):
                        if q_dirty[q]:
                            nc.gpsimd.trigger_dma(count=None, queue_num=q)
                            q_dirty[q] = []

                nG = len(GROUPS)
                if PREP:
                    PD = LAG
                    for gj in range(min(PD, nG)):
                        prep_call(gj, 0)
                        prep_call(gj, 1)
                    for gj in range(min(3, nG)):
                        issue_ptt(gj)
                    for gi in range(nG):
                        # Pool is in-order: every issued prep has retired
                        # before a later trigger executes, so firing all
                        # dirty queues is safe.
                        trigger_dirty()
                        if gi + PD < nG:
                            prep_call(gi + PD, 0)
                            prep_call(gi + PD, 1)
                        _process_group(gi)
                        if ag_after and gi in ag_after:
                            ag_after[gi]()
                        if gi + 3 < nG:
                            issue_ptt(gi + 3)
                    return
                for gi in range(nG + LAG):
                    if gi < nG:
                        pend[gi] = {0: issue_gather(gi, 0)}
                    ok = gi - (LAG - 2)
                    if 0 <= ok < nG:
                        pend[ok] = pend.get(ok, {})
                        pend[ok][1] = issue_gather(ok, 1)
                        issue_ptt(ok)
                    # odd-stride bump so the parity->queue mapping rotates
                    # (4 calls/iter on 4 queues otherwise pins p0 to q0/q1)
                    qn[0] += 1
                    pk = gi - LAG
                    if 0 <= pk < nG:
                        _process_group(pk)
                        if ag_after and pk in ag_after:
                            ag_after[pk]()

            # AG2 fired right after the last group's g_sh write
            def ag2():
                ov2 = g_fl[:].rearrange("(c r) f -> c r f", c=CORES)
                nc.gpsimd.collective_compute(
                    "AllGather", mybir.AluOpType.bypass,
                    replica_groups=[list(range(CORES))],
                    ins=[g_sh[:].opt()],
                    outs=[ov2.opt()])

            spmm_pass(hcat_fl, 256, 256, out1_d, True, 1,
                      ag_after={len(GROUPS) - 1: ag2})
            spmm_pass(g_fl, 128, 128, out2_d, False, 2)

    nc.compile()
    return nc


def _prepare_inputs(x, W, b, plan):
    relabel = plan["relabel"]
    xpad = np.zeros((NP, C), np.float32)
    xpad[relabel[:N]] = x
    xT = xpad.T
    Wp = np.concatenate([W[0], W[1], W[2]], axis=1)
    biasrow = np.zeros((128, 384), np.float32)
    biasrow[0] = np.concatenate([b[0], b[1], b[2]])
    wb = np.concatenate([Wp, biasrow], axis=1)

    in_maps = []
    for c in range(CORES):
        in_maps.append({
            "xT": np.ascontiguousarray(
                xT[:, c * RPC:(c + 1) * RPC]).astype(ml_dtypes.bfloat16),
            "wb": wb.astype(ml_dtypes.bfloat16),
            "rs": plan["rs_tab"][c][None, :].astype(ml_dtypes.bfloat16),
            "ptt": plan["pt"][c],
            "gixt": plan["gidx_w"][c],
        })
    return in_maps


def kernel(x, W, b, edge_val, edge_row, edge_col):
    x = np.asarray(x, np.float32)
    W = np.asarray(W, np.float32)
    b = np.asarray(b, np.float32)
    edge_val = np.asarray(edge_val, np.float32)
    edge_row = np.asarray(edge_row, np.int32)
    edge_col = np.asarray(edge_col, np.int32)

    from concourse.bass_utils import run_bass_kernel_spmd

    key = hash((edge_row.tobytes(), edge_col.tobytes(), edge_val.tobytes()))
    if key not in _CACHE:
        plan = _build_plan(edge_row, edge_col, edge_val)
        nc = _build_program(plan)
        _CACHE[key] = (plan, nc)
    plan, nc = _CACHE[key]

    in_maps = _prepare_inputs(x, W, b, plan)
    res = run_bass_kernel_spmd(nc, in_maps, core_ids=list(range(CORES)),
                               trace=TRACE)
    kernel.last_results = res
    parts = []
    for c in range(CORES):
        r = res.results[c]
        blk = np.stack([np.asarray(r["out0"], np.float32),
                        np.asarray(r["out1"], np.float32),
                        np.asarray(r["out2"], np.float32)], axis=-2)
        parts.append(blk.transpose(1, 0, 2, 3).reshape(RPC, 384))
    full = np.concatenate(parts, axis=0)
    return np.ascontiguousarray(full[plan["relabel"][:N]])


kernel.last_results = None

if __name__ == "__main__":
    rng = np.random.default_rng(0)
    x = rng.standard_normal((N, C), dtype=np.float32)
    W = rng.standard_normal((3, C, C), dtype=np.float32) / np.sqrt(C)
    b = rng.standard_normal((3, C), dtype=np.float32) * 0.01
    ev = rng.random(E, dtype=np.float32)
    er = rng.integers(0, N, E, dtype=np.int32)
    ec = rng.integers(0, N, E, dtype=np.int32)
    out = kernel(x=x, W=W, b=b, edge_val=ev, edge_row=er, edge_col=ec)
    print(out.shape, out.dtype)
